# revision 5
# baseline (speedup 1.0000x reference)
"""Trainium2 Bass kernel for the NeuralODE (Tsit5, linear-in-t vector field) problem.

The reference integrates dy/dt = f(t) = t * w with Tsit5 on a fixed grid
ts[k] = k/T.  Because f is independent of y and linear in t, the Tsit5 update
collapses to y[k] = y0 + 0.5*ts[k]^2 * w (the 5th-order method integrates a
degree-1 polynomial exactly; with ts[k] = k*2^-12 the closed form
0.5*ts[k]^2 = k^2 * 2^-25 is exactly representable in fp32).

Kernel strategy (per core, 8-way shard over the state dim D=8192 -> 1024):
  out[k, d] = y0[d] + a[k] * w[d],   a[k] = 0.5 * ts[k]^2
  - ts loaded as (128, 32) SBUF tile: [p, f] = ts[p*32 + f]
  - k-tiles are columns j: k = p*32 + j  (a per-partition scalar per tile)
  - ScalarE: prod = w_bcast * a[:, j]  (activation Copy with per-partition scale)
  - VectorE: out_tile_slice = prod + y0_bcast
  - 8 output DMAs of (128, 4096) = 2 MiB each; rows p*32+g*4+jj are 4
    consecutive DRAM rows -> 16 KiB contiguous per partition descriptor.
"""

import numpy as np

_T = 4096
_D = 8192
_NCORES = 8
_DS = _D // _NCORES  # 1024 state elements per core
_P = 128
_F = _T // _P        # 32 time columns
_JJ = 4              # k-tiles per output super-tile
_G = _F // _JJ       # 8 output super-tiles

_CACHE = {}


def _program(repeat=None):
    """Build (and cache) the Bass program. repeat=None emits the kernel body
    once; repeat=N wraps it in an on-device For_i loop (benchmarking only)."""
    key = ("nc", repeat)
    if key in _CACHE:
        return _CACHE[key]
    import concourse.bacc as bacc
    import concourse.mybir as mybir
    from concourse.tile import TileContext

    f32 = mybir.dt.float32
    nc = bacc.Bacc("TRN2", target_bir_lowering=False, debug=False)
    ts_d = nc.declare_dram_parameter("ts", [_T], f32, isOutput=False)
    y0_d = nc.declare_dram_parameter("y0s", [_DS], f32, isOutput=False)
    w_d = nc.declare_dram_parameter("ws", [_DS], f32, isOutput=False)
    out_d = nc.declare_dram_parameter("out", [_T, _DS], f32, isOutput=True)

    def body(tc, const_pool, prod_pool, big_pool):
        ts_sb = const_pool.tile([_P, _F], f32)
        nc.sync.dma_start(out=ts_sb[:], in_=ts_d[:].rearrange("(p f) -> p f", p=_P))
        a_sb = const_pool.tile([_P, _F], f32)
        nc.vector.tensor_mul(out=a_sb[:], in0=ts_sb[:], in1=ts_sb[:])
        nc.vector.tensor_scalar_mul(a_sb[:], a_sb[:], 0.5)

        w_tile = const_pool.tile([_P, _DS], f32)
        nc.gpsimd.dma_start(
            out=w_tile[:], in_=w_d[:].unsqueeze(0).to_broadcast((_P, _DS))
        )
        y0_tile = const_pool.tile([_P, _DS], f32)
        nc.gpsimd.dma_start(
            out=y0_tile[:], in_=y0_d[:].unsqueeze(0).to_broadcast((_P, _DS))
        )

        # out_view[g][p, jj*DS + d] = out[p*32 + g*JJ + jj, d]
        out_view = out_d[:].rearrange("(p g jj) d -> g p (jj d)", p=_P, g=_G, jj=_JJ)
        for g in range(_G):
            big = big_pool.tile([_P, _JJ * _DS], f32)
            for jj in range(_JJ):
                j = g * _JJ + jj
                prod = prod_pool.tile([_P, _DS], f32)
                nc.scalar.activation(
                    prod[:],
                    w_tile[:],
                    mybir.ActivationFunctionType.Copy,
                    bias=0.0,
                    scale=a_sb[:, j : j + 1],
                )
                nc.vector.tensor_add(
                    out=big[:, jj * _DS : (jj + 1) * _DS],
                    in0=prod[:],
                    in1=y0_tile[:],
                )
            nc.sync.dma_start(out=out_view[g], in_=big[:])

    with TileContext(nc) as tc:
        with (
            tc.tile_pool(name="const", bufs=1) as const_pool,
            tc.tile_pool(name="prod", bufs=6) as prod_pool,
            tc.tile_pool(name="big", bufs=3) as big_pool,
        ):
            if repeat is None:
                body(tc, const_pool, prod_pool, big_pool)
            else:
                with tc.For_i(0, repeat, 1):
                    body(tc, const_pool, prod_pool, big_pool)

    nc.compile()
    _CACHE[key] = nc
    return nc


def _run(ts, y0, W, trace=False):
    ts = np.ascontiguousarray(np.asarray(ts, dtype=np.float32))
    y0 = np.ascontiguousarray(np.asarray(y0, dtype=np.float32))
    W = np.ascontiguousarray(np.asarray(W, dtype=np.float32))
    assert ts.shape == (_T,) and y0.shape == (_D,) and W.shape == (1, _D)

    nc = _program()
    from concourse.bass_utils import run_bass_kernel_spmd

    in_maps = [
        {
            "ts": ts,
            "y0s": y0[i * _DS : (i + 1) * _DS],
            "ws": W[0, i * _DS : (i + 1) * _DS],
        }
        for i in range(_NCORES)
    ]
    res = run_bass_kernel_spmd(nc, in_maps, list(range(_NCORES)), trace=trace)
    out = np.concatenate([res.results[i]["out"] for i in range(_NCORES)], axis=1)
    return out, res


def kernel(ts, y0, W):
    out, _ = _run(ts, y0, W, trace=False)
    return out


# revision 12
# speedup vs baseline: 2.9526x; 2.9526x over previous
"""Trainium2 Bass kernel for the NeuralODE (Tsit5, linear-in-t vector field) problem.

The reference integrates dy/dt = f(t) = t * w with Tsit5 on a fixed grid
ts[k] = k/T.  Because f is independent of y and linear in t, the Tsit5 update
collapses to y[k] = y0 + 0.5*ts[k]^2 * w (the 5th-order method integrates a
degree-1 polynomial exactly; with ts[k] = k*2^-12 the closed form
0.5*ts[k]^2 = k^2 * 2^-25 is exactly representable in fp32).

Kernel strategy (per core, 8-way shard over the state dim D=8192 -> 1024):
  out[k, d] = y0[d] + a[k] * w[d],   a[k] = 0.5 * ts[k]^2
  - ts loaded as (128, 32) SBUF tile: [p, f] = ts[p*32 + f]
  - k-tiles are columns j: k = p*32 + j  (a per-partition scalar per tile)
  - w/y0 broadcast across partitions via PE matmul with a ones vector
    (a stride-0 broadcast DMA re-reads one HBM line 128x and is ~5 us
    per tensor due to bank contention; PE does it in ~1 us)
  - ScalarE: prod = w_bcast * a[:, j]  (activation Copy, per-partition scale)
  - VectorE: out_slice = prod + y0_bcast
  - output DMAs in ragged groups of k-tiles (first/last small so the DMA
    stream starts early and ends with a short tail); rows p*32+j for
    consecutive j are consecutive DRAM rows -> contiguous per-partition
    descriptors of sz*4 KiB.
"""

import numpy as np

_T = 4096
_D = 8192
_NCORES = 8
_DS = _D // _NCORES  # 1024 state elements per core
_P = 128
_F = _T // _P  # 32 time columns (k-tiles)

_GROUPS = [1, 1, 2, 4, 4, 4, 4, 4, 4, 2, 1, 1]  # k-tiles per output DMA
assert sum(_GROUPS) == _F

_CACHE = {}


def _program(repeat=None, variant="full"):
    """Build (and cache) the Bass program. repeat=None emits the kernel body
    once; repeat=N wraps it in an on-device For_i loop (benchmarking only).

    variant (bench ablations):
      full        - the real kernel (PE broadcast, ragged groups)
      swdge_bcast - broadcast via stride-0 SWDGE DMA (old method)
      even_groups - 8 groups of 4 k-tiles
      no_dve      - ACT writes big slices directly, no add
      no_act      - DVE adds w_tile+y0_tile directly, no ACT mult
      no_dma      - compute only, skip the output DMAs
      dma_only    - output DMAs of big tiles filled once by ACT
      no_bcast    - broadcasts replaced by memset
      empty       - trivial body (loop overhead measurement)
    """
    key = ("nc", repeat, variant)
    if key in _CACHE:
        return _CACHE[key]
    import concourse.bacc as bacc
    import concourse.bass as bass
    import concourse.mybir as mybir
    from concourse.tile import TileContext

    f32 = mybir.dt.float32
    nc = bacc.Bacc("TRN2", target_bir_lowering=False, debug=False)
    ts_d = nc.declare_dram_parameter("ts", [_T], f32, isOutput=False)
    y0_d = nc.declare_dram_parameter("y0s", [_DS], f32, isOutput=False)
    w_d = nc.declare_dram_parameter("ws", [_DS], f32, isOutput=False)
    out_d = nc.declare_dram_parameter("out", [_T, _DS], f32, isOutput=True)

    if variant == "even_groups":
        groups = [4] * 8
    elif variant == "groups9":
        groups = [2, 2, 4, 4, 4, 4, 4, 4, 4]
    elif variant == "groups16":
        groups = [2] * 16
    else:
        groups = _GROUPS

    def body(tc, const_pool, prod_pool, big_pool, psum_pool):
        if variant == "empty":
            tiny = const_pool.tile([_P, _F], f32)
            nc.vector.memset(tiny[:], 0.0)
            return

        w_tile = const_pool.tile([_P, _DS], f32)
        y0_tile = const_pool.tile([_P, _DS], f32)
        if variant not in ("no_bcast", "swdge_bcast"):
            # PE broadcast: out(128, n) = ones(1,128).T @ row(1, n).
            # Emitted first: the w path gates the whole compute stream.
            ones_row = const_pool.tile([1, _P], f32)
            nc.vector.memset(ones_row[:], 1.0)
            w_row = const_pool.tile([1, _DS], f32)
            nc.sync.dma_start(out=w_row[:], in_=w_d[:].unsqueeze(0))
            y0_row = const_pool.tile([1, _DS], f32)
            nc.sync.dma_start(out=y0_row[:], in_=y0_d[:].unsqueeze(0))
            nmm = _DS // 512
            for h in range(nmm):
                sl = slice(h * 512, (h + 1) * 512)
                pw = psum_pool.tile([_P, 512], f32)
                nc.tensor.matmul(
                    pw[:], ones_row[:], w_row[:, sl], start=True, stop=True
                )
                # DVE copies: the ACT table load then overlaps the broadcast
                # instead of gating the first w chunk.
                nc.vector.tensor_copy(out=w_tile[:, sl], in_=pw[:])
            for h in range(nmm):
                sl = slice(h * 512, (h + 1) * 512)
                py = psum_pool.tile([_P, 512], f32)
                nc.tensor.matmul(
                    py[:], ones_row[:], y0_row[:, sl], start=True, stop=True
                )
                nc.vector.tensor_copy(out=y0_tile[:, sl], in_=py[:])

        ts_sb = const_pool.tile([_P, _F], f32)
        nc.sync.dma_start(out=ts_sb[:], in_=ts_d[:].rearrange("(p f) -> p f", p=_P))
        a_sb = const_pool.tile([_P, _F], f32)
        nc.vector.tensor_mul(out=a_sb[:], in0=ts_sb[:], in1=ts_sb[:])
        nc.vector.tensor_scalar_mul(a_sb[:], a_sb[:], 0.5)

        if variant == "no_bcast":
            nc.vector.memset(w_tile[:], 1.0)
            nc.vector.memset(y0_tile[:], 0.5)
        elif variant == "swdge_bcast":
            nc.gpsimd.dma_start(
                out=w_tile[:], in_=w_d[:].unsqueeze(0).to_broadcast((_P, _DS))
            )
            nc.gpsimd.dma_start(
                out=y0_tile[:], in_=y0_d[:].unsqueeze(0).to_broadcast((_P, _DS))
            )

        # out_flat[p, j*DS + d] = out[p*32 + j, d]
        out_flat = out_d[:].rearrange("(p j) d -> p (j d)", p=_P)
        off = 0
        for sz in groups:
            big = big_pool.tile([_P, 4 * _DS], f32)
            if variant == "dma_only":
                nc.scalar.activation(
                    big[:, 0:_DS],
                    w_tile[:],
                    mybir.ActivationFunctionType.Copy,
                    bias=0.0,
                    scale=a_sb[:, 0:1],
                )
                nc.sync.dma_start(
                    out=out_flat[:, off * _DS : (off + sz) * _DS],
                    in_=big[:, 0 : sz * _DS],
                )
                off += sz
                continue
            for jj in range(sz):
                j = off + jj
                sl = big[:, jj * _DS : (jj + 1) * _DS]
                if variant == "no_act":
                    nc.vector.tensor_add(out=sl, in0=w_tile[:], in1=y0_tile[:])
                    continue
                if variant == "no_dve":
                    nc.scalar.activation(
                        sl,
                        w_tile[:],
                        mybir.ActivationFunctionType.Copy,
                        bias=0.0,
                        scale=a_sb[:, j : j + 1],
                    )
                    continue
                prod = prod_pool.tile([_P, _DS], f32)
                nc.scalar.activation(
                    prod[:],
                    w_tile[:],
                    mybir.ActivationFunctionType.Copy,
                    bias=0.0,
                    scale=a_sb[:, j : j + 1],
                )
                nc.vector.tensor_add(out=sl, in0=prod[:], in1=y0_tile[:])
            if variant != "no_dma":
                nc.sync.dma_start(
                    out=out_flat[:, off * _DS : (off + sz) * _DS],
                    in_=big[:, 0 : sz * _DS],
                )
            off += sz

    with TileContext(nc) as tc:
        with (
            tc.tile_pool(name="const", bufs=1) as const_pool,
            tc.tile_pool(name="prod", bufs=8) as prod_pool,
            tc.tile_pool(name="big", bufs=6) as big_pool,
            tc.tile_pool(name="psum", bufs=2, space="PSUM") as psum_pool,
        ):
            if repeat is None:
                body(tc, const_pool, prod_pool, big_pool, psum_pool)
            else:
                with tc.For_i(0, repeat, 1):
                    body(tc, const_pool, prod_pool, big_pool, psum_pool)

    nc.compile()
    _CACHE[key] = nc
    return nc


def _run(ts, y0, W, trace=False):
    ts = np.ascontiguousarray(np.asarray(ts, dtype=np.float32))
    y0 = np.ascontiguousarray(np.asarray(y0, dtype=np.float32))
    W = np.ascontiguousarray(np.asarray(W, dtype=np.float32))
    assert ts.shape == (_T,) and y0.shape == (_D,) and W.shape == (1, _D)

    nc = _program()
    from concourse.bass_utils import run_bass_kernel_spmd

    in_maps = [
        {
            "ts": ts,
            "y0s": y0[i * _DS : (i + 1) * _DS],
            "ws": W[0, i * _DS : (i + 1) * _DS],
        }
        for i in range(_NCORES)
    ]
    res = run_bass_kernel_spmd(nc, in_maps, list(range(_NCORES)), trace=trace)
    out = np.concatenate([res.results[i]["out"] for i in range(_NCORES)], axis=1)
    return out, res


def kernel(ts, y0, W):
    out, _ = _run(ts, y0, W, trace=False)
    return out


# revision 18
# speedup vs baseline: 3.1597x; 1.0701x over previous
"""Trainium2 Bass kernel for the NeuralODE (Tsit5, linear-in-t vector field) problem.

The reference integrates dy/dt = f(t) = t * w with Tsit5 on a fixed grid
ts[k] = k/T.  Because f is independent of y and linear in t, the Tsit5 update
collapses to y[k] = y0 + 0.5*ts[k]^2 * w (the 5th-order method integrates a
degree-1 polynomial exactly; with ts[k] = k*2^-12 the closed form
0.5*ts[k]^2 = k^2 * 2^-25 is exactly representable in fp32).

Kernel strategy (per core, 8-way shard over the state dim D=8192 -> 1024):
  out[k, d] = y0[d] + a[k] * w[d],   a[k] = 0.5 * ts[k]^2
  - ts loaded as (128, 32) SBUF tile: [p, f] = ts[p*32 + f]
  - k-tiles are columns j: k = p*32 + j  (a per-partition scalar per tile)
  - w/y0 broadcast across partitions via PE matmul with a ones vector
    (a stride-0 broadcast DMA re-reads one HBM line 128x and is ~5 us
    per tensor due to bank contention; PE does it in ~1 us)
  - ScalarE: prod = w_bcast * a[:, j]  (activation Copy, per-partition scale)
  - VectorE: out_slice = prod + y0_bcast
  - output DMAs in ragged groups of k-tiles (first/last small so the DMA
    stream starts early and ends with a short tail); rows p*32+j for
    consecutive j are consecutive DRAM rows -> contiguous per-partition
    descriptors of sz*4 KiB.
"""

import numpy as np

_T = 4096
_D = 8192
_NCORES = 8
_DS = _D // _NCORES  # 1024 state elements per core
_P = 128
_F = _T // _P  # 32 time columns (k-tiles)

_GROUPS = [1, 1, 2, 4, 4, 4, 4, 4, 4, 2, 1, 1]  # k-tiles per output DMA
assert sum(_GROUPS) == _F

_CACHE = {}


def _program(repeat=None, variant="full"):
    """Build (and cache) the Bass program. repeat=None emits the kernel body
    once; repeat=N wraps it in an on-device For_i loop (benchmarking only).

    variant (bench ablations):
      full        - the real kernel (PE broadcast, ragged groups)
      swdge_bcast - broadcast via stride-0 SWDGE DMA (old method)
      even_groups - 8 groups of 4 k-tiles
      no_dve      - ACT writes big slices directly, no add
      no_act      - DVE adds w_tile+y0_tile directly, no ACT mult
      no_dma      - compute only, skip the output DMAs
      dma_only    - output DMAs of big tiles filled once by ACT
      no_bcast    - broadcasts replaced by memset
      empty       - trivial body (loop overhead measurement)
    """
    key = ("nc", repeat, variant)
    if key in _CACHE:
        return _CACHE[key]
    import concourse.bacc as bacc
    import concourse.bass as bass
    import concourse.mybir as mybir
    from concourse.tile import TileContext

    f32 = mybir.dt.float32
    nc = bacc.Bacc("TRN2", target_bir_lowering=False, debug=False)
    ts_d = nc.declare_dram_parameter("ts", [_T], f32, isOutput=False)
    y0_d = nc.declare_dram_parameter("y0s", [_DS], f32, isOutput=False)
    w_d = nc.declare_dram_parameter("ws", [_DS], f32, isOutput=False)
    out_d = nc.declare_dram_parameter("out", [_T, _DS], f32, isOutput=True)

    if variant == "even_groups":
        groups = [4] * 8
    elif variant == "groups9":
        groups = [2, 2, 4, 4, 4, 4, 4, 4, 4]
    elif variant == "groups16":
        groups = [2] * 16
    elif variant == "groups13":
        groups = [1, 1, 2, 2, 4, 4, 4, 4, 4, 2, 2, 1, 1]
    else:
        groups = _GROUPS
    assert sum(groups) == _F

    def body(tc, const_pool, prod_pool, big_pool, psum_pool, wpsum_pool):
        if variant == "empty":
            tiny = const_pool.tile([_P, _F], f32)
            nc.vector.memset(tiny[:], 0.0)
            return

        w_tile = const_pool.tile([_P, _DS], f32)
        y0_tile = const_pool.tile([_P, _DS], f32)
        w_src = w_tile
        if variant not in ("no_bcast", "swdge_bcast"):
            # PE broadcast: out(128, n) = ones(1,128).T @ row(1, n).
            # Emitted first: the w path gates the whole compute stream.
            ones_row = const_pool.tile([1, _P], f32)
            nc.vector.memset(ones_row[:], 1.0)
            w_row = const_pool.tile([1, _DS], f32)
            nc.sync.dma_start(out=w_row[:], in_=w_d[:].unsqueeze(0))
            y0_row = const_pool.tile([1, _DS], f32)
            nc.sync.dma_start(out=y0_row[:], in_=y0_d[:].unsqueeze(0))
            nmm = _DS // 512
            if variant == "wpsum":
                # Keep broadcast w resident in PSUM; ACT reads it directly
                # (faster PSUM-src fixed cost, one less hop on the head).
                w_ps = wpsum_pool.tile([_P, _DS], f32)
                for h in range(nmm):
                    sl = slice(h * 512, (h + 1) * 512)
                    nc.tensor.matmul(
                        w_ps[:, sl], ones_row[:], w_row[:, sl], start=True, stop=True
                    )
                w_src = w_ps
            else:
                for h in range(nmm):
                    sl = slice(h * 512, (h + 1) * 512)
                    pw = psum_pool.tile([_P, 512], f32)
                    nc.tensor.matmul(
                        pw[:], ones_row[:], w_row[:, sl], start=True, stop=True
                    )
                    # DVE copies: the ACT table load then overlaps the broadcast
                    # instead of gating the first w chunk.
                    if variant == "actcopy":
                        nc.scalar.copy(w_tile[:, sl], pw[:])
                    else:
                        nc.vector.tensor_copy(out=w_tile[:, sl], in_=pw[:])
            for h in range(nmm):
                sl = slice(h * 512, (h + 1) * 512)
                py = psum_pool.tile([_P, 512], f32)
                nc.tensor.matmul(
                    py[:], ones_row[:], y0_row[:, sl], start=True, stop=True
                )
                if variant == "actcopy":
                    nc.scalar.copy(y0_tile[:, sl], py[:])
                else:
                    nc.vector.tensor_copy(out=y0_tile[:, sl], in_=py[:])

        ts_sb = const_pool.tile([_P, _F], f32)
        nc.sync.dma_start(out=ts_sb[:], in_=ts_d[:].rearrange("(p f) -> p f", p=_P))
        a_sb = const_pool.tile([_P, _F], f32)
        nc.vector.tensor_mul(out=a_sb[:], in0=ts_sb[:], in1=ts_sb[:])
        nc.vector.tensor_scalar_mul(a_sb[:], a_sb[:], 0.5)

        if variant == "no_bcast":
            nc.vector.memset(w_tile[:], 1.0)
            nc.vector.memset(y0_tile[:], 0.5)
        elif variant == "swdge_bcast":
            nc.gpsimd.dma_start(
                out=w_tile[:], in_=w_d[:].unsqueeze(0).to_broadcast((_P, _DS))
            )
            nc.gpsimd.dma_start(
                out=y0_tile[:], in_=y0_d[:].unsqueeze(0).to_broadcast((_P, _DS))
            )

        # out_flat[p, j*DS + d] = out[p*32 + j, d]
        out_flat = out_d[:].rearrange("(p j) d -> p (j d)", p=_P)
        off = 0
        for gi, sz in enumerate(groups):
            dma_eng = nc.scalar if (variant == "dualring" and gi % 2) else nc.sync
            big = big_pool.tile([_P, 4 * _DS], f32)
            if variant == "dma_only":
                nc.scalar.activation(
                    big[:, 0:_DS],
                    w_src[:],
                    mybir.ActivationFunctionType.Copy,
                    bias=0.0,
                    scale=a_sb[:, 0:1],
                )
                dma_eng.dma_start(
                    out=out_flat[:, off * _DS : (off + sz) * _DS],
                    in_=big[:, 0 : sz * _DS],
                )
                off += sz
                continue
            for jj in range(sz):
                j = off + jj
                sl = big[:, jj * _DS : (jj + 1) * _DS]
                if variant == "no_act":
                    nc.vector.tensor_add(out=sl, in0=w_tile[:], in1=y0_tile[:])
                    continue
                if variant == "no_dve":
                    nc.scalar.activation(
                        sl,
                        w_src[:],
                        mybir.ActivationFunctionType.Copy,
                        bias=0.0,
                        scale=a_sb[:, j : j + 1],
                    )
                    continue
                prod = prod_pool.tile([_P, _DS], f32)
                nc.scalar.activation(
                    prod[:],
                    w_src[:],
                    mybir.ActivationFunctionType.Copy,
                    bias=0.0,
                    scale=a_sb[:, j : j + 1],
                )
                nc.vector.tensor_add(out=sl, in0=prod[:], in1=y0_tile[:])
            if variant != "no_dma":
                dma_eng.dma_start(
                    out=out_flat[:, off * _DS : (off + sz) * _DS],
                    in_=big[:, 0 : sz * _DS],
                )
            off += sz

    with TileContext(nc) as tc:
        with (
            tc.tile_pool(name="const", bufs=1) as const_pool,
            tc.tile_pool(name="prod", bufs=10 if variant == "bufs8" else 8) as prod_pool,
            tc.tile_pool(name="big", bufs=8 if variant == "bufs8" else 6) as big_pool,
            tc.tile_pool(name="psum", bufs=2, space="PSUM") as psum_pool,
            tc.tile_pool(name="wpsum", bufs=1, space="PSUM") as wpsum_pool,
        ):
            if repeat is None:
                body(tc, const_pool, prod_pool, big_pool, psum_pool, wpsum_pool)
            else:
                with tc.For_i(0, repeat, 1):
                    body(tc, const_pool, prod_pool, big_pool, psum_pool, wpsum_pool)

    nc.compile()
    _CACHE[key] = nc
    return nc


def _run(ts, y0, W, trace=False):
    ts = np.ascontiguousarray(np.asarray(ts, dtype=np.float32))
    y0 = np.ascontiguousarray(np.asarray(y0, dtype=np.float32))
    W = np.ascontiguousarray(np.asarray(W, dtype=np.float32))
    assert ts.shape == (_T,) and y0.shape == (_D,) and W.shape == (1, _D)

    nc = _program()
    from concourse.bass_utils import run_bass_kernel_spmd

    in_maps = [
        {
            "ts": ts,
            "y0s": y0[i * _DS : (i + 1) * _DS],
            "ws": W[0, i * _DS : (i + 1) * _DS],
        }
        for i in range(_NCORES)
    ]
    res = run_bass_kernel_spmd(nc, in_maps, list(range(_NCORES)), trace=trace)
    out = np.concatenate([res.results[i]["out"] for i in range(_NCORES)], axis=1)
    return out, res


def kernel(ts, y0, W):
    out, _ = _run(ts, y0, W, trace=False)
    return out
